# revision 11
# baseline (speedup 1.0000x reference)
"""Trainium2 Bass kernel for nn_DecomposingAttnProcessor.

Math (reference):
    q = hs @ Wq.T + bq;  k = ehs @ Wk.T + bk;  v = ehs @ Wv.T + bv
    scores = (q @ k.T) * dh**-0.5 per (bc, head)      [BC, H, S, T]
    w = softmax(scores over the COMPONENT axis)        (bc = c*B + b, C=4, B=2)
    w = w / (sum_t w + eps)
    out = (w @ v) -> [BC, S, D] -> @ Wo.T + bo

Distribution: shard S (4096 query tokens) across 8 cores, 512 each. Every
core handles all (bc, head) pairs for its S-slice, so the component softmax
group (same b, all c) stays on one core and each core emits complete output
rows (no cross-core reduction).

v2 design:
  - all matmul operands bf16 (FWL-capable weight loads, half DMA/SBUF),
    fp32 PSUM accumulation, fp32 biases.
  - rowsum folded into the AV matmul: V tiles carry a 65th ones-column per
    head so po[64] = sum_t w; normalize = DVE recip + GpSimd
    partition_broadcast + DVE mul.  No separate ones-matmuls.
  - V projection packed densely over all bc (1232 rows, chunks cross bc
    boundaries) instead of per-(bc, T-chunk) padded matmuls.
  - program emission interleaves the Q projection of batch b=1 into the
    attention loop of b=0, and the O projection of b=0 into attention of
    b=1, so the PE stays warm and busy through the attention phase.
"""

import numpy as np
from contextlib import ExitStack

import concourse.bass as bass
import concourse.tile as tile
from concourse import bacc, mybir

F32 = mybir.dt.float32
BF16 = mybir.dt.bfloat16

# problem shape (hardcoded per contract)
BC, S, D = 8, 4096, 1536
T = 154
C, B = 4, 2
H, DH = 24, 64
NCORES = 8
SL = S // NCORES          # 512 S-rows per core
NDI = D // 128            # 12 din chunks
NDO = D // 128            # 12 dout tiles
KVN = BC * T              # 1232 packed kv rows (no padding)
T0, T1 = 128, T - 128     # T chunks: 128 + 26
SCALE = DH ** -0.5
HPP = H // 2              # 12 head pairs
W2 = 2 * SL               # 1024: free width of a (c) block (both heads)

KV_NBLK = [(0, 512), (512, 512), (1024, 208)]    # N-blocks for kT proj
DO_NBLK = [(0, 512), (512, 512), (1024, 512)]    # N-blocks over D for v proj
VCHUNKS = [(i * 128, min(128, KVN - i * 128)) for i in range((KVN + 127) // 128)]


def build_program():
    nc = bacc.Bacc("TRN2", target_bir_lowering=False, debug=False)

    # ---- external I/O (per core) ----
    hsT = nc.dram_tensor("hsT", [BC, D, SL], BF16, kind="ExternalInput").ap()
    ehsT = nc.dram_tensor("ehsT", [D, KVN], BF16, kind="ExternalInput").ap()
    wqT = nc.dram_tensor("wqT", [D, D], BF16, kind="ExternalInput").ap()
    wkT = nc.dram_tensor("wkT", [D, D], BF16, kind="ExternalInput").ap()
    wvT = nc.dram_tensor("wvT", [D, D], BF16, kind="ExternalInput").ap()
    woT = nc.dram_tensor("woT", [D, D], BF16, kind="ExternalInput").ap()
    # biases laid out [128, 12] on host (bq pre-scaled by dh**-0.5)
    bqs = nc.dram_tensor("bqs", [128, NDO], F32, kind="ExternalInput").ap()
    bks = nc.dram_tensor("bks", [128, NDO], F32, kind="ExternalInput").ap()
    bos = nc.dram_tensor("bos", [128, NDO], F32, kind="ExternalInput").ap()
    bvr = nc.dram_tensor("bvr", [1, D], BF16, kind="ExternalInput").ap()
    onesr = nc.dram_tensor("onesr", [1, 128], BF16, kind="ExternalInput").ap()
    outT = nc.dram_tensor("outT", [BC, D, SL], F32, kind="ExternalOutput").ap()

    # ---- DRAM scratch ----
    import os
    _dbg = os.environ.get("DK_DEBUG", "0") == "1"
    _kind = dict(kind="ExternalOutput") if _dbg else {}
    kT_s = [nc.dram_tensor(f"kT_s{j}", [128, KVN], BF16, **_kind).ap()
            for j in range(HPP)]
    v_s = nc.dram_tensor("v_s", [KVN, D], BF16, **_kind).ap()
    qT_s = [nc.dram_tensor(f"qT_s{bc}", [NDO, 128, SL], BF16, **_kind).ap()
            for bc in range(BC)]
    at_s = [nc.dram_tensor(f"at_s{bc}", [D, SL], BF16, **_kind).ap()
            for bc in range(BC)]
    if _dbg:
        po_dbg = nc.dram_tensor("po_dbg", [1, 2 * SL], F32,
                                kind="ExternalOutput").ap()
        e0_dbg = nc.dram_tensor("e0_dbg", [T0, C * 2 * SL], BF16,
                                kind="ExternalOutput").ap()
        dd_dbg = nc.dram_tensor("dd_dbg", [T0, 2 * SL], F32,
                                kind="ExternalOutput").ap()
        rr_dbg = nc.dram_tensor("rr_dbg", [1, 2 * SL], F32,
                                kind="ExternalOutput").ap()
        rb_dbg = nc.dram_tensor("rb_dbg", [64, 2 * SL], F32,
                                kind="ExternalOutput").ap()
        vt_dbg = nc.dram_tensor("vt_dbg", [T0, 130], BF16,
                                kind="ExternalOutput").ap()

    with tile.TileContext(nc) as tc, ExitStack() as ctx:
        # ---------- persistent pools ----------
        const = ctx.enter_context(tc.tile_pool(name="const", bufs=1))
        # PSUM pools: s0 2x1bk + s1 1bk + po 3x1bk + proj 2x1bk = 8 banks
        ps_s0 = ctx.enter_context(tc.tile_pool(name="ps_s0", bufs=2,
                                               space="PSUM"))
        ps_s1 = ctx.enter_context(tc.tile_pool(name="ps_s1", bufs=1,
                                               space="PSUM"))
        ps_po = ctx.enter_context(tc.tile_pool(name="ps_po", bufs=3,
                                               space="PSUM"))
        ps_pj = ctx.enter_context(tc.tile_pool(name="ps_pj", bufs=2,
                                               space="PSUM"))

        # ---------- constants ----------
        bq_t = const.tile([128, NDO], F32)
        bk_t = const.tile([128, NDO], F32)
        bo_t = const.tile([128, NDO], F32)
        bv_t = const.tile([1, D], BF16)
        ones_row = const.tile([1, 128], BF16)
        nc.sync.dma_start(bq_t[:], bqs[:])
        nc.sync.dma_start(bk_t[:], bks[:])
        nc.sync.dma_start(bo_t[:], bos[:])
        nc.sync.dma_start(bv_t[:], bvr[:])
        nc.sync.dma_start(ones_row[:], onesr[:])

        # ================= P1: K and V projections =================
        with tc.tile_pool(name="p1w", bufs=1) as p1w, \
             tc.tile_pool(name="p1o", bufs=2) as p1o:
            eh = p1w.tile([128, NDI * KVN], BF16, name="eh")
            wk = p1w.tile([128, NDI * D], BF16, name="wk")
            wv = p1w.tile([128, NDI * D], BF16, name="wv")
            nc.sync.dma_start(
                eh[:], ehsT[:].rearrange("(i p) n -> p i n", p=128))
            nc.sync.dma_start(
                wk[:], wkT[:].rearrange("(i p) n -> p i n", p=128))
            nc.sync.dma_start(
                wv[:], wvT[:].rearrange("(i p) n -> p i n", p=128))

            # ---- P1a: kT = Wk @ ehs^T (+bk), all bc packed ----
            for j in range(HPP):          # 12 head-pair row blocks
                ot = p1o.tile([128, KVN], BF16, tag="kot")
                for (nb0, nbl) in KV_NBLK:
                    ps = ps_pj.tile([128, 512], F32, tag="ps")
                    for i in range(NDI):
                        nc.tensor.matmul(
                            ps[:, 0:nbl],
                            wk[:, i * D + j * 128:i * D + (j + 1) * 128],
                            eh[:, i * KVN + nb0:i * KVN + nb0 + nbl],
                            start=(i == 0), stop=(i == NDI - 1))
                    nc.scalar.activation(
                        ot[:, nb0:nb0 + nbl], ps[:, 0:nbl],
                        mybir.ActivationFunctionType.Identity,
                        bias=bk_t[:, j:j + 1])
                nc.sync.dma_start(kT_s[j][:], ot[:])

            # ---- P1b: v = ehs @ Wv.T (+bv), rows packed across bc ----
            for (ck0, ckl) in VCHUNKS:
                ot = p1o.tile([128, D], BF16, tag="vot")
                for (nb0, nbl) in DO_NBLK:
                    ps = ps_pj.tile([128, 512], F32, tag="ps")
                    for i in range(NDI):
                        nc.tensor.matmul(
                            ps[0:ckl, :],
                            eh[:, i * KVN + ck0:i * KVN + ck0 + ckl],
                            wv[:, i * D + nb0:i * D + nb0 + nbl],
                            start=(i == 0), stop=False)
                    nc.tensor.matmul(
                        ps[0:ckl, :],
                        ones_row[0:1, 0:ckl],
                        bv_t[0:1, nb0:nb0 + nbl],
                        start=False, stop=True,
                        skip_group_check=True)
                    nc.scalar.copy(ot[0:ckl, nb0:nb0 + nbl], ps[0:ckl, :])
                nc.sync.dma_start(v_s[ck0:ck0 + ckl, :], ot[0:ckl, :])

        # ---------- P3 pools (reuse P1's SBUF space) ----------
        p3qk = ctx.enter_context(tc.tile_pool(name="p3qk", bufs=2))
        p3v = ctx.enter_context(tc.tile_pool(name="p3v", bufs=1))
        p3e = ctx.enter_context(tc.tile_pool(name="p3e", bufs=2))
        p3d = ctx.enter_context(tc.tile_pool(name="p3d", bufs=2))
        p3at = ctx.enter_context(tc.tile_pool(name="p3at", bufs=2))

        # persistent V tiles: [T', 128] = [v (64 cols) | ones (64 cols)] per
        # (component, head-in-pair, hp-parity double buffer). The ones block
        # makes the AV matmul emit the row-sum replicated on out partitions
        # 64:128, so the key renormalize is a single DVE divide (no
        # partition broadcast).
        vt0 = [[[p3v.tile([T0, 128], BF16, tag=f"vt0_{c}_{k}_{p}",
                          name=f"vt0_{c}_{k}_{p}") for p in range(2)]
                for k in range(2)] for c in range(C)]
        vt1 = [[[p3v.tile([T1, 128], BF16, tag=f"vt1_{c}_{k}_{p}",
                          name=f"vt1_{c}_{k}_{p}") for p in range(2)]
                for k in range(2)] for c in range(C)]
        for c in range(C):
            for k in range(2):
                for p in range(2):
                    nc.vector.memset(vt0[c][k][p][:, 64:128], 1.0)
                    nc.vector.memset(vt1[c][k][p][:, 64:128], 1.0)

        # ================= P2/P4 emitters =================
        def emit_qproj(wq, hpool, opool, bc):
            ht = hpool.tile([128, NDI * SL], BF16, tag="ht")
            nc.sync.dma_start(
                ht[:], hsT[bc].rearrange("(i p) n -> p i n", p=128))
            for j in range(NDO):
                ps = ps_pj.tile([128, SL], F32, tag="ps")
                for i in range(NDI):
                    nc.tensor.matmul(
                        ps[:], wq[:, i * D + j * 128:i * D + (j + 1) * 128],
                        ht[:, i * SL:(i + 1) * SL],
                        start=(i == 0), stop=(i == NDI - 1))
                qt = opool.tile([128, SL], BF16, tag="qt")
                nc.scalar.activation(
                    qt[:], ps[:], mybir.ActivationFunctionType.Identity,
                    bias=bq_t[:, j:j + 1], scale=SCALE)
                nc.sync.dma_start(qT_s[bc][j], qt[:])

        def emit_oproj(wo, apool, opool, bc):
            att = apool.tile([128, NDI * SL], BF16, tag="att")
            nc.sync.dma_start(
                att[:], at_s[bc][:].rearrange("(i p) n -> p i n", p=128))
            for j in range(NDO):
                ps = ps_pj.tile([128, SL], F32, tag="ps")
                for i in range(NDI):
                    nc.tensor.matmul(
                        ps[:], wo[:, i * D + j * 128:i * D + (j + 1) * 128],
                        att[:, i * SL:(i + 1) * SL],
                        start=(i == 0), stop=(i == NDI - 1))
                ot = opool.tile([128, SL], F32, tag="ot")
                nc.scalar.activation(
                    ot[:], ps[:], mybir.ActivationFunctionType.Identity,
                    bias=bo_t[:, j:j + 1])
                nc.sync.dma_start(outT[bc][j * 128:(j + 1) * 128, :], ot[:])

        # ================= P3 emitter =================
        MULT = mybir.AluOpType.mult
        ADD = mybir.AluOpType.add
        DIV = mybir.AluOpType.divide

        def emit_attn(b, hp):
            par = hp % 2
            qt, kt = [], []
            for c in range(C):
                bc = c * B + b
                for j in range(2):
                    q = p3qk.tile([64, SL], BF16, tag=f"q{c}{j}")
                    nc.sync.dma_start(
                        q[:], qT_s[bc][hp][j * 64:(j + 1) * 64, :])
                    qt.append(q)
                    k = p3qk.tile([64, T], BF16, tag=f"k{c}{j}")
                    nc.sync.dma_start(
                        k[:], kT_s[hp][j * 64:(j + 1) * 64,
                                       bc * T:(bc + 1) * T])
                    kt.append(k)
                    # v for this (component, head): cols 0:64 of the vt
                    # tiles (cols 64:128 stay ones)
                    nc.sync.dma_start(
                        vt0[c][j][par][:, 0:64],
                        v_s[bc * T:bc * T + T0,
                            (hp * 2 + j) * 64:(hp * 2 + j + 1) * 64])
                    nc.sync.dma_start(
                        vt1[c][j][par][:, 0:64],
                        v_s[bc * T + T0:bc * T + T,
                            (hp * 2 + j) * 64:(hp * 2 + j + 1) * 64])

            # scores + exp; e layout [T', C*W2] = (c, j, s) c-major
            e0 = p3e.tile([T0, C * W2], BF16, tag="e0")
            e1 = p3e.tile([T1, C * W2], BF16, tag="e1")
            for c in range(C):
                for j in range(2):
                    qk, kk = qt[c * 2 + j], kt[c * 2 + j]
                    sl_e = slice(c * W2 + j * SL, c * W2 + (j + 1) * SL)
                    s0 = ps_s0.tile([T0, SL], F32, tag="s0")
                    nc.tensor.matmul(s0[:], kk[:, 0:T0], qk[:],
                                     start=True, stop=True)
                    nc.scalar.activation(
                        e0[:, sl_e], s0[:],
                        mybir.ActivationFunctionType.Exp)
                    s1 = ps_s1.tile([T1, SL], F32, tag="s1")
                    nc.tensor.matmul(s1[:], kk[:, T0:T], qk[:],
                                     start=True, stop=True)
                    nc.scalar.activation(
                        e1[:, sl_e], s1[:],
                        mybir.ActivationFunctionType.Exp)

            # component softmax: d = sum_c e_c, e_c *= 1/d.  bf16 stt adds
            # run in DVE 4x perf mode; the f32 steps (sum + cast) ride on
            # gpsimd, reciprocal on DVE (f32-only custom op).
            for (ee, rows) in ((e0, T0), (e1, T1)):
                d01 = p3d.tile([rows, W2], BF16, tag=f"d01_{rows}")
                d23 = p3d.tile([rows, W2], BF16, tag=f"d23_{rows}")
                ddf = p3d.tile([rows, W2], F32, tag=f"ddf_{rows}")
                ddb = p3d.tile([rows, W2], BF16, tag=f"ddb_{rows}")
                nc.vector.scalar_tensor_tensor(
                    d01[:], ee[:, 0:W2], 1.0, ee[:, W2:2 * W2], MULT, ADD)
                nc.vector.scalar_tensor_tensor(
                    d23[:], ee[:, 2 * W2:3 * W2], 1.0, ee[:, 3 * W2:4 * W2],
                    MULT, ADD)
                nc.gpsimd.tensor_add(ddf[:], d01[:], d23[:])
                nc.vector.reciprocal_approx_fast(ddf[:], ddf[:])
                nc.gpsimd.tensor_copy(ddb[:], ddf[:])
                for c in range(C):
                    sl_ = slice(c * W2, (c + 1) * W2)
                    nc.vector.scalar_tensor_tensor(
                        ee[:, sl_], ee[:, sl_], 1.0, ddb[:], MULT, MULT)

            # AV with folded rowsum (ones block -> po rows 64:128);
            # renormalize: ACT cross-base copy of the rowsum block, DVE
            # reciprocal, one stt multiply (single PSUM input, base 0).
            for c in range(C):
                bc = c * B + b
                for j in range(2):
                    esl = slice(c * W2 + j * SL, c * W2 + (j + 1) * SL)
                    po = ps_po.tile([128, SL], F32, tag="po")
                    nc.tensor.matmul(po[:], vt0[c][j][par][:], e0[:, esl],
                                     start=True, stop=False)
                    nc.tensor.matmul(po[:], vt1[c][j][par][:], e1[:, esl],
                                     start=False, stop=True)
                    rsr = p3d.tile([64, SL], F32, tag="rsr", bufs=3)
                    nc.scalar.copy(rsr[:], po[64:128, :])
                    nc.vector.reciprocal_approx_fast(rsr[:], rsr[:])
                    at = p3at.tile([64, SL], BF16, tag="at", bufs=3)
                    nc.vector.scalar_tensor_tensor(
                        at[:], po[0:64, :], 1.0, rsr[:], MULT, MULT)
                    h = hp * 2 + j
                    nc.sync.dma_start(
                        at_s[bc][h * 64:(h + 1) * 64, :], at[:])

        # ================= schedule =================
        with tc.tile_pool(name="p2w", bufs=1) as p2w, \
             tc.tile_pool(name="p2h", bufs=2) as p2h, \
             tc.tile_pool(name="p2o", bufs=3) as p2o:
            wq = p2w.tile([128, NDI * D], BF16, name="wq")
            nc.sync.dma_start(
                wq[:], wqT[:].rearrange("(i p) n -> p i n", p=128))
            for bc in (0, 2, 4, 6):
                emit_qproj(wq, p2h, p2o, bc)
            # window 2: attention b=0 interleaved with Q proj b=1
            for hp in range(HPP):
                if hp % 3 == 0:
                    emit_qproj(wq, p2h, p2o, (1, 3, 5, 7)[hp // 3])
                emit_attn(0, hp)

        with tc.tile_pool(name="p4w", bufs=1) as p4w, \
             tc.tile_pool(name="p4a", bufs=2) as p4a, \
             tc.tile_pool(name="p4o", bufs=3) as p4o:
            wo = p4w.tile([128, NDI * D], BF16, name="wo")
            nc.sync.dma_start(
                wo[:], woT[:].rearrange("(i p) n -> p i n", p=128))
            # window 3: attention b=1 interleaved with O proj b=0
            for hp in range(HPP):
                emit_attn(1, hp)
                if hp % 3 == 2:
                    emit_oproj(wo, p4a, p4o, (0, 2, 4, 6)[hp // 3])
            for bc in (1, 3, 5, 7):
                emit_oproj(wo, p4a, p4o, bc)

    nc.compile()
    return nc


_NC_CACHE = None


def _get_program():
    global _NC_CACHE
    if _NC_CACHE is None:
        _NC_CACHE = build_program()
    return _NC_CACHE


def make_in_maps(hidden_states, encoder_hidden_states, Wq, bq, Wk, bk,
                 Wv, bv, Wo, bo):
    """Host-side shard + transpose prep. Returns per-core input dicts."""
    import ml_dtypes
    bf16 = ml_dtypes.bfloat16
    hs = np.ascontiguousarray(hidden_states, dtype=np.float32)
    ehs = np.ascontiguousarray(encoder_hidden_states, dtype=np.float32)

    # ehsT [D, KVN]: all bc packed contiguously, no padding
    ehsT = np.ascontiguousarray(
        ehs.transpose(2, 0, 1).reshape(D, KVN)).astype(bf16)

    shared = {
        "ehsT": ehsT,
        "wqT": np.ascontiguousarray(Wq.T).astype(bf16),
        "wkT": np.ascontiguousarray(Wk.T).astype(bf16),
        "wvT": np.ascontiguousarray(Wv.T).astype(bf16),
        "woT": np.ascontiguousarray(Wo.T).astype(bf16),
        "bqs": np.ascontiguousarray(
            (np.asarray(bq, np.float32) * SCALE).reshape(NDO, 128).T),
        "bks": np.ascontiguousarray(
            np.asarray(bk, np.float32).reshape(NDO, 128).T),
        "bos": np.ascontiguousarray(
            np.asarray(bo, np.float32).reshape(NDO, 128).T),
        "bvr": np.asarray(bv, np.float32).reshape(1, D).astype(bf16),
        "onesr": np.ones((1, 128), np.float32).astype(bf16),
    }
    in_maps = []
    for core in range(NCORES):
        sl = slice(core * SL, (core + 1) * SL)
        hsT = np.ascontiguousarray(hs[:, sl, :].transpose(0, 2, 1)
                                   ).astype(bf16)
        in_maps.append({**shared, "hsT": hsT})
    return in_maps


def run_sharded(inputs, trace=False, tmpdir=None, trace_cores=None):
    from concourse.bass_utils import run_bass_kernel_spmd
    nc = _get_program()
    in_maps = make_in_maps(**inputs)
    res = run_bass_kernel_spmd(nc, in_maps, list(range(NCORES)), trace=trace,
                               tmpdir=tmpdir, trace_cores=trace_cores)
    out = np.empty((BC, S, D), dtype=np.float32)
    for core in range(NCORES):
        sl = slice(core * SL, (core + 1) * SL)
        out[:, sl, :] = res.results[core]["outT"].transpose(0, 2, 1)
    return out, res


def kernel(**inputs):
    out, _ = run_sharded(inputs, trace=False)
    return out



# revision 15
# speedup vs baseline: 1.0787x; 1.0787x over previous
"""Trainium2 Bass kernel for nn_DecomposingAttnProcessor.

Math (reference):
    q = hs @ Wq.T + bq;  k = ehs @ Wk.T + bk;  v = ehs @ Wv.T + bv
    scores = (q @ k.T) * dh**-0.5 per (bc, head)      [BC, H, S, T]
    w = softmax(scores over the COMPONENT axis)        (bc = c*B + b, C=4, B=2)
    w = w / (sum_t w + eps)
    out = (w @ v) -> [BC, S, D] -> @ Wo.T + bo

Distribution: shard S (4096 query tokens) across 8 cores, 512 each. Every
core handles all (bc, head) pairs for its S-slice, so the component softmax
group (same b, all c) stays on one core and each core emits complete output
rows (no cross-core reduction).

v2 design:
  - all matmul operands bf16 (FWL-capable weight loads, half DMA/SBUF),
    fp32 PSUM accumulation, fp32 biases.
  - rowsum folded into the AV matmul: V tiles carry a 65th ones-column per
    head so po[64] = sum_t w; normalize = DVE recip + GpSimd
    partition_broadcast + DVE mul.  No separate ones-matmuls.
  - V projection packed densely over all bc (1232 rows, chunks cross bc
    boundaries) instead of per-(bc, T-chunk) padded matmuls.
  - program emission interleaves the Q projection of batch b=1 into the
    attention loop of b=0, and the O projection of b=0 into attention of
    b=1, so the PE stays warm and busy through the attention phase.
"""

import numpy as np
from contextlib import ExitStack

import concourse.bass as bass
import concourse.tile as tile
from concourse import bacc, mybir

F32 = mybir.dt.float32
BF16 = mybir.dt.bfloat16

# problem shape (hardcoded per contract)
BC, S, D = 8, 4096, 1536
T = 154
C, B = 4, 2
H, DH = 24, 64
NCORES = 8
SL = S // NCORES          # 512 S-rows per core
NDI = D // 128            # 12 din chunks
NDO = D // 128            # 12 dout tiles
KVN = BC * T              # 1232 packed kv rows (no padding)
T0, T1 = 128, T - 128     # T chunks: 128 + 26
SCALE = DH ** -0.5
HPP = H // 2              # 12 head pairs
W2 = 2 * SL               # 1024: free width of a (c) block (both heads)

KV_NBLK = [(0, 512), (512, 512), (1024, 208)]    # N-blocks for kT proj
DO_NBLK = [(0, 512), (512, 512), (1024, 512)]    # N-blocks over D for v proj
VCHUNKS = [(i * 128, min(128, KVN - i * 128)) for i in range((KVN + 127) // 128)]


def build_program():
    nc = bacc.Bacc("TRN2", target_bir_lowering=False, debug=False)

    # ---- external I/O (per core) ----
    hsT = nc.dram_tensor("hsT", [BC, D, SL], BF16, kind="ExternalInput").ap()
    ehsT = nc.dram_tensor("ehsT", [D, KVN], BF16, kind="ExternalInput").ap()
    wqT = nc.dram_tensor("wqT", [D, D], BF16, kind="ExternalInput").ap()
    wkT = nc.dram_tensor("wkT", [D, D], BF16, kind="ExternalInput").ap()
    wvT = nc.dram_tensor("wvT", [D, D], BF16, kind="ExternalInput").ap()
    woT = nc.dram_tensor("woT", [D, D], BF16, kind="ExternalInput").ap()
    # biases laid out [128, 12] on host (bq pre-scaled by dh**-0.5)
    bqs = nc.dram_tensor("bqs", [128, NDO], F32, kind="ExternalInput").ap()
    bks = nc.dram_tensor("bks", [128, NDO], F32, kind="ExternalInput").ap()
    bos = nc.dram_tensor("bos", [128, NDO], F32, kind="ExternalInput").ap()
    bvr = nc.dram_tensor("bvr", [1, D], BF16, kind="ExternalInput").ap()
    onesr = nc.dram_tensor("onesr", [1, 128], BF16, kind="ExternalInput").ap()
    outT = nc.dram_tensor("outT", [BC, D, SL], F32, kind="ExternalOutput").ap()

    # ---- DRAM scratch ----
    import os
    _dbg = os.environ.get("DK_DEBUG", "0") == "1"
    _kind = dict(kind="ExternalOutput") if _dbg else {}
    kT_s = [nc.dram_tensor(f"kT_s{j}", [128, KVN], BF16, **_kind).ap()
            for j in range(HPP)]
    v_s = nc.dram_tensor("v_s", [KVN, D], BF16, **_kind).ap()
    qT_s = [nc.dram_tensor(f"qT_s{bc}", [NDO, 128, SL], BF16, **_kind).ap()
            for bc in range(BC)]
    at_s = [nc.dram_tensor(f"at_s{bc}", [D, SL], BF16, **_kind).ap()
            for bc in range(BC)]
    if _dbg:
        po_dbg = nc.dram_tensor("po_dbg", [1, 2 * SL], F32,
                                kind="ExternalOutput").ap()
        e0_dbg = nc.dram_tensor("e0_dbg", [T0, C * 2 * SL], BF16,
                                kind="ExternalOutput").ap()
        dd_dbg = nc.dram_tensor("dd_dbg", [T0, 2 * SL], F32,
                                kind="ExternalOutput").ap()
        rr_dbg = nc.dram_tensor("rr_dbg", [1, 2 * SL], F32,
                                kind="ExternalOutput").ap()
        rb_dbg = nc.dram_tensor("rb_dbg", [64, 2 * SL], F32,
                                kind="ExternalOutput").ap()
        vt_dbg = nc.dram_tensor("vt_dbg", [T0, 130], BF16,
                                kind="ExternalOutput").ap()

    with tile.TileContext(nc) as tc, ExitStack() as ctx:
        # ---------- persistent pools ----------
        const = ctx.enter_context(tc.tile_pool(name="const", bufs=1))
        # PSUM pools: s0 2x1bk + s1 1bk + po 3x1bk + proj 2x1bk = 8 banks
        ps_s0 = ctx.enter_context(tc.tile_pool(name="ps_s0", bufs=2,
                                               space="PSUM"))
        ps_s1 = ctx.enter_context(tc.tile_pool(name="ps_s1", bufs=1,
                                               space="PSUM"))
        ps_po = ctx.enter_context(tc.tile_pool(name="ps_po", bufs=3,
                                               space="PSUM"))
        ps_pj = ctx.enter_context(tc.tile_pool(name="ps_pj", bufs=2,
                                               space="PSUM"))

        # ---------- constants ----------
        bq_t = const.tile([128, NDO], F32)
        bk_t = const.tile([128, NDO], F32)
        bo_t = const.tile([128, NDO], F32)
        bv_t = const.tile([1, D], BF16)
        ones_row = const.tile([1, 128], BF16)
        nc.sync.dma_start(bq_t[:], bqs[:])
        nc.sync.dma_start(bk_t[:], bks[:])
        nc.sync.dma_start(bo_t[:], bos[:])
        nc.sync.dma_start(bv_t[:], bvr[:])
        nc.sync.dma_start(ones_row[:], onesr[:])

        # ================= P1: K and V projections =================
        with tc.tile_pool(name="p1w", bufs=1) as p1w, \
             tc.tile_pool(name="p1o", bufs=2) as p1o:
            eh = p1w.tile([128, NDI * KVN], BF16, name="eh")
            wk = p1w.tile([128, NDI * D], BF16, name="wk")
            wv = p1w.tile([128, NDI * D], BF16, name="wv")
            nc.sync.dma_start(
                eh[:], ehsT[:].rearrange("(i p) n -> p i n", p=128))
            nc.sync.dma_start(
                wk[:], wkT[:].rearrange("(i p) n -> p i n", p=128))
            nc.sync.dma_start(
                wv[:], wvT[:].rearrange("(i p) n -> p i n", p=128))

            # ---- P1a: kT = Wk @ ehs^T (+bk), all bc packed ----
            for j in range(HPP):          # 12 head-pair row blocks
                ot = p1o.tile([128, KVN], BF16, tag="kot")
                for (nb0, nbl) in KV_NBLK:
                    ps = ps_pj.tile([128, 512], F32, tag="ps")
                    for i in range(NDI):
                        nc.tensor.matmul(
                            ps[:, 0:nbl],
                            wk[:, i * D + j * 128:i * D + (j + 1) * 128],
                            eh[:, i * KVN + nb0:i * KVN + nb0 + nbl],
                            start=(i == 0), stop=(i == NDI - 1))
                    nc.scalar.activation(
                        ot[:, nb0:nb0 + nbl], ps[:, 0:nbl],
                        mybir.ActivationFunctionType.Identity,
                        bias=bk_t[:, j:j + 1])
                nc.sync.dma_start(kT_s[j][:], ot[:])

            # ---- P1b: v = ehs @ Wv.T (+bv), rows packed across bc ----
            for (ck0, ckl) in VCHUNKS:
                ot = p1o.tile([128, D], BF16, tag="vot")
                for (nb0, nbl) in DO_NBLK:
                    ps = ps_pj.tile([128, 512], F32, tag="ps")
                    for i in range(NDI):
                        nc.tensor.matmul(
                            ps[0:ckl, :],
                            eh[:, i * KVN + ck0:i * KVN + ck0 + ckl],
                            wv[:, i * D + nb0:i * D + nb0 + nbl],
                            start=(i == 0), stop=False)
                    nc.tensor.matmul(
                        ps[0:ckl, :],
                        ones_row[0:1, 0:ckl],
                        bv_t[0:1, nb0:nb0 + nbl],
                        start=False, stop=True,
                        skip_group_check=True)
                    nc.scalar.copy(ot[0:ckl, nb0:nb0 + nbl], ps[0:ckl, :])
                nc.sync.dma_start(v_s[ck0:ck0 + ckl, :], ot[0:ckl, :])

        # ---------- P3 pools (reuse P1's SBUF space) ----------
        p3qk = ctx.enter_context(tc.tile_pool(name="p3qk", bufs=2))
        p3v = ctx.enter_context(tc.tile_pool(name="p3v", bufs=1))
        p3e = ctx.enter_context(tc.tile_pool(name="p3e", bufs=2))
        p3d = ctx.enter_context(tc.tile_pool(name="p3d", bufs=2))
        p3at = ctx.enter_context(tc.tile_pool(name="p3at", bufs=2))

        # persistent V tiles: [T', 128] = [v (64 cols) | ones (64 cols)] per
        # (component, head-in-pair, hp-parity double buffer). The ones block
        # makes the AV matmul emit the row-sum replicated on out partitions
        # 64:128, so the key renormalize is a single DVE divide (no
        # partition broadcast).
        vt0 = [[[p3v.tile([T0, 128], BF16, tag=f"vt0_{c}_{k}_{p}",
                          name=f"vt0_{c}_{k}_{p}") for p in range(2)]
                for k in range(2)] for c in range(C)]
        vt1 = [[[p3v.tile([T1, 128], BF16, tag=f"vt1_{c}_{k}_{p}",
                          name=f"vt1_{c}_{k}_{p}") for p in range(2)]
                for k in range(2)] for c in range(C)]
        for c in range(C):
            for k in range(2):
                for p in range(2):
                    nc.vector.memset(vt0[c][k][p][:, 64:128], 1.0)
                    nc.vector.memset(vt1[c][k][p][:, 64:128], 1.0)

        # ================= P2/P4 emitters =================
        def emit_qproj(wq, hpool, opool, bc):
            ht = hpool.tile([128, NDI * SL], BF16, tag="ht")
            nc.sync.dma_start(
                ht[:], hsT[bc].rearrange("(i p) n -> p i n", p=128))
            for j in range(NDO):
                ps = ps_pj.tile([128, SL], F32, tag="ps")
                for i in range(NDI):
                    nc.tensor.matmul(
                        ps[:], wq[:, i * D + j * 128:i * D + (j + 1) * 128],
                        ht[:, i * SL:(i + 1) * SL],
                        start=(i == 0), stop=(i == NDI - 1))
                qt = opool.tile([128, SL], BF16, tag="qt")
                nc.scalar.activation(
                    qt[:], ps[:], mybir.ActivationFunctionType.Identity,
                    bias=bq_t[:, j:j + 1], scale=SCALE)
                nc.sync.dma_start(qT_s[bc][j], qt[:])

        def emit_oproj(wo, apool, opool, bc):
            att = apool.tile([128, NDI * SL], BF16, tag="att")
            nc.sync.dma_start(
                att[:], at_s[bc][:].rearrange("(i p) n -> p i n", p=128))
            for j in range(NDO):
                ps = ps_pj.tile([128, SL], F32, tag="ps")
                for i in range(NDI):
                    nc.tensor.matmul(
                        ps[:], wo[:, i * D + j * 128:i * D + (j + 1) * 128],
                        att[:, i * SL:(i + 1) * SL],
                        start=(i == 0), stop=(i == NDI - 1))
                ot = opool.tile([128, SL], F32, tag="ot")
                nc.scalar.activation(
                    ot[:], ps[:], mybir.ActivationFunctionType.Identity,
                    bias=bo_t[:, j:j + 1])
                nc.sync.dma_start(outT[bc][j * 128:(j + 1) * 128, :], ot[:])

        # ================= P3 emitter =================
        MULT = mybir.AluOpType.mult
        ADD = mybir.AluOpType.add
        DIV = mybir.AluOpType.divide

        def emit_attn(b, hp):
            par = hp % 2
            qt, kt = [], []
            for c in range(C):
                bc = c * B + b
                for j in range(2):
                    q = p3qk.tile([64, SL], BF16, tag=f"q{c}{j}")
                    nc.sync.dma_start(
                        q[:], qT_s[bc][hp][j * 64:(j + 1) * 64, :])
                    qt.append(q)
                    k = p3qk.tile([64, T], BF16, tag=f"k{c}{j}")
                    nc.sync.dma_start(
                        k[:], kT_s[hp][j * 64:(j + 1) * 64,
                                       bc * T:(bc + 1) * T])
                    kt.append(k)
                    # v for this (component, head): cols 0:64 of the vt
                    # tiles (cols 64:128 stay ones)
                    nc.sync.dma_start(
                        vt0[c][j][par][:, 0:64],
                        v_s[bc * T:bc * T + T0,
                            (hp * 2 + j) * 64:(hp * 2 + j + 1) * 64])
                    nc.sync.dma_start(
                        vt1[c][j][par][:, 0:64],
                        v_s[bc * T + T0:bc * T + T,
                            (hp * 2 + j) * 64:(hp * 2 + j + 1) * 64])

            # scores + exp; e layout [T', C*W2] = (c, j, s) c-major
            e0 = p3e.tile([T0, C * W2], BF16, tag="e0")
            e1 = p3e.tile([T1, C * W2], BF16, tag="e1")
            for c in range(C):
                for j in range(2):
                    qk, kk = qt[c * 2 + j], kt[c * 2 + j]
                    sl_e = slice(c * W2 + j * SL, c * W2 + (j + 1) * SL)
                    s0 = ps_s0.tile([T0, SL], F32, tag="s0")
                    nc.tensor.matmul(s0[:], kk[:, 0:T0], qk[:],
                                     start=True, stop=True)
                    nc.scalar.activation(
                        e0[:, sl_e], s0[:],
                        mybir.ActivationFunctionType.Exp)
                    s1 = ps_s1.tile([T1, SL], F32, tag="s1")
                    nc.tensor.matmul(s1[:], kk[:, T0:T], qk[:],
                                     start=True, stop=True)
                    nc.scalar.activation(
                        e1[:, sl_e], s1[:],
                        mybir.ActivationFunctionType.Exp)

            # component softmax: d = sum_c e_c, e_c *= 1/d.  bf16 TT adds run
            # in the DVE 2x datapath; the f32 sum rides on gpsimd,
            # reciprocal on DVE (f32-only custom op); the normalize muls are
            # mixed-dtype TT (bf16 x f32 -> bf16), split vector/gpsimd.
            for (ee, rows) in ((e0, T0), (e1, T1)):
                d01 = p3d.tile([rows, W2], BF16, tag=f"d01_{rows}")
                d23 = p3d.tile([rows, W2], BF16, tag=f"d23_{rows}")
                ddf = p3d.tile([rows, W2], F32, tag=f"ddf_{rows}")
                nc.vector.tensor_add(d01[:], ee[:, 0:W2], ee[:, W2:2 * W2])
                nc.vector.tensor_add(d23[:], ee[:, 2 * W2:3 * W2],
                                     ee[:, 3 * W2:4 * W2])
                nc.gpsimd.tensor_add(ddf[:], d01[:], d23[:])
                nc.vector.reciprocal_approx_fast(ddf[:], ddf[:])
                for c in range(C):
                    sl_ = slice(c * W2, (c + 1) * W2)
                    nc.vector.tensor_mul(ee[:, sl_], ee[:, sl_], ddf[:])

            # AV with folded rowsum (ones block -> po rows 64:128).
            # Renormalize: ACT cross-base drain of the rowsum block, DVE
            # reciprocal, one TT multiply (single PSUM input, base 0).
            for c in range(C):
                bc = c * B + b
                for j in range(2):
                    esl = slice(c * W2 + j * SL, c * W2 + (j + 1) * SL)
                    po = ps_po.tile([128, SL], F32, tag="po")
                    nc.tensor.matmul(po[:], vt0[c][j][par][:], e0[:, esl],
                                     start=True, stop=False)
                    nc.tensor.matmul(po[:], vt1[c][j][par][:], e1[:, esl],
                                     start=False, stop=True)
                    rsr = p3d.tile([64, SL], F32, tag="rsr", bufs=3)
                    nc.scalar.copy(rsr[:], po[64:128, :])
                    nc.vector.reciprocal_approx_fast(rsr[:], rsr[:])
                    at = p3at.tile([64, SL], BF16, tag="at", bufs=3)
                    nc.vector.tensor_mul(at[:], po[0:64, :], rsr[:])
                    h = hp * 2 + j
                    nc.sync.dma_start(
                        at_s[bc][h * 64:(h + 1) * 64, :], at[:])

        # ================= schedule =================
        with tc.tile_pool(name="p2w", bufs=1) as p2w, \
             tc.tile_pool(name="p2h", bufs=2) as p2h, \
             tc.tile_pool(name="p2o", bufs=3) as p2o:
            wq = p2w.tile([128, NDI * D], BF16, name="wq")
            nc.sync.dma_start(
                wq[:], wqT[:].rearrange("(i p) n -> p i n", p=128))
            for bc in (0, 2, 4, 6):
                emit_qproj(wq, p2h, p2o, bc)
            # window 2: attention b=0 interleaved with Q proj b=1
            for hp in range(HPP):
                if hp % 3 == 0:
                    emit_qproj(wq, p2h, p2o, (1, 3, 5, 7)[hp // 3])
                emit_attn(0, hp)

        with tc.tile_pool(name="p4w", bufs=1) as p4w, \
             tc.tile_pool(name="p4a", bufs=2) as p4a, \
             tc.tile_pool(name="p4o", bufs=3) as p4o:
            wo = p4w.tile([128, NDI * D], BF16, name="wo")
            nc.sync.dma_start(
                wo[:], woT[:].rearrange("(i p) n -> p i n", p=128))
            # window 3: attention b=1 interleaved with O proj b=0
            for hp in range(HPP):
                emit_attn(1, hp)
                if hp % 3 == 2:
                    emit_oproj(wo, p4a, p4o, (0, 2, 4, 6)[hp // 3])
            for bc in (1, 3, 5, 7):
                emit_oproj(wo, p4a, p4o, bc)

    nc.compile()
    return nc


_NC_CACHE = None


def _get_program():
    global _NC_CACHE
    if _NC_CACHE is None:
        _NC_CACHE = build_program()
    return _NC_CACHE


def make_in_maps(hidden_states, encoder_hidden_states, Wq, bq, Wk, bk,
                 Wv, bv, Wo, bo):
    """Host-side shard + transpose prep. Returns per-core input dicts."""
    import ml_dtypes
    bf16 = ml_dtypes.bfloat16
    hs = np.ascontiguousarray(hidden_states, dtype=np.float32)
    ehs = np.ascontiguousarray(encoder_hidden_states, dtype=np.float32)

    # ehsT [D, KVN]: all bc packed contiguously, no padding
    ehsT = np.ascontiguousarray(
        ehs.transpose(2, 0, 1).reshape(D, KVN)).astype(bf16)

    shared = {
        "ehsT": ehsT,
        "wqT": np.ascontiguousarray(Wq.T).astype(bf16),
        "wkT": np.ascontiguousarray(Wk.T).astype(bf16),
        "wvT": np.ascontiguousarray(Wv.T).astype(bf16),
        "woT": np.ascontiguousarray(Wo.T).astype(bf16),
        "bqs": np.ascontiguousarray(
            (np.asarray(bq, np.float32) * SCALE).reshape(NDO, 128).T),
        "bks": np.ascontiguousarray(
            np.asarray(bk, np.float32).reshape(NDO, 128).T),
        "bos": np.ascontiguousarray(
            np.asarray(bo, np.float32).reshape(NDO, 128).T),
        "bvr": np.asarray(bv, np.float32).reshape(1, D).astype(bf16),
        "onesr": np.ones((1, 128), np.float32).astype(bf16),
    }
    in_maps = []
    for core in range(NCORES):
        sl = slice(core * SL, (core + 1) * SL)
        hsT = np.ascontiguousarray(hs[:, sl, :].transpose(0, 2, 1)
                                   ).astype(bf16)
        in_maps.append({**shared, "hsT": hsT})
    return in_maps


def run_sharded(inputs, trace=False, tmpdir=None, trace_cores=None):
    from concourse.bass_utils import run_bass_kernel_spmd
    nc = _get_program()
    in_maps = make_in_maps(**inputs)
    res = run_bass_kernel_spmd(nc, in_maps, list(range(NCORES)), trace=trace,
                               tmpdir=tmpdir, trace_cores=trace_cores)
    out = np.empty((BC, S, D), dtype=np.float32)
    for core in range(NCORES):
        sl = slice(core * SL, (core + 1) * SL)
        out[:, sl, :] = res.results[core]["outT"].transpose(0, 2, 1)
    return out, res


def kernel(**inputs):
    out, _ = run_sharded(inputs, trace=False)
    return out

